# revision 1
# baseline (speedup 1.0000x reference)
"""Trainium2 Bass kernel for nn_CombinedLoss (retrieval_knn).

Computes:
  loss = 0.5*chamfer(pc1_0, pc2) + 0.5*chamfer(pc1_1, pc2)
       + 0.5*mean((pc1_3 - conf(pc3, pc2))^2) + mean((pc1_0 - pc2)^2)

Strategy (per spec sharding hint):
  - Chamfer query rows sharded across 8 cores; each core holds the full
    reference cloud pc2 (16384 x 3).
  - Device computes all O(N^2) pairwise-min work; host does only O(N)
    post-processing (cross-core min combine, sqrt, means).

Device kernel (per core):
  - d2 entries produced by the PE as K=20 bf16 hi/lo matmuls:
    alpha = [-2a, 1, |a|^2], beta = [b, |b|^2, 1], each split into
    bf16 hi+lo and arranged so alpha_aug . beta_aug reproduces the exact
    fp32 product sum (ah*bh + al*bh + ah*bl + al*bl).
  - References (pc2) on output partitions (stationary side), queries on
    the moving free axis.
  - ScalarE evacuates PSUM f32 -> SBUF fp16.
  - VectorE: tensor_scalar w/ min-accum (4x mode) gives per-reference
    min over this core's query shard ("col" direction);
    tensor_tensor min (2x mode) accumulates the per-query running min
    across reference tiles ("row" direction).
"""

import sys

sys.path.insert(0, "/opt/trn_rl_repo")

import numpy as np
import ml_dtypes

from concourse import bass, bacc, mybir, tile
from concourse.bass_utils import run_bass_kernel_spmd

BF16 = ml_dtypes.bfloat16

N_CORES = 8
B, M, S, N = 8, 2048, 512, 256
NB = B * M          # 16384 reference points (pc2 flattened)
NA = B * M          # 16384 cd query points (pc1_0 flattened)
NS = B * S          # 4096 seed query points (pc1_1 flattened)
A_SH = NA // N_CORES   # 2048 cd queries per core
S_SH = NS // N_CORES   # 512 seed queries per core
NT = NB // 128         # 128 reference tiles

ALPHA = 0.5
BETA = 0.5


def _hilo(x):
    """f32 [5, n] -> (hi, lo) bf16 arrays with x ~= hi + lo exactly split."""
    hi = x.astype(BF16)
    lo = (x - hi.astype(np.float32)).astype(BF16)
    return hi, lo


def _aug_moving(pts):
    """alpha side: [-2p, 1, |p|^2] -> [20, n] bf16 (hi,lo,hi,lo)."""
    n = pts.shape[0]
    a = np.empty((5, n), np.float32)
    a[0:3] = -2.0 * pts.T
    a[3] = 1.0
    a[4] = (pts.astype(np.float32) ** 2).sum(1)
    hi, lo = _hilo(a)
    return np.concatenate([hi, lo, hi, lo], 0)


def _aug_stationary(pts):
    """beta side: [p, |p|^2, 1] -> [20, n] bf16 (hi,hi,lo,lo)."""
    n = pts.shape[0]
    b = np.empty((5, n), np.float32)
    b[0:3] = pts.T
    b[3] = (pts.astype(np.float32) ** 2).sum(1)
    b[4] = 1.0
    hi, lo = _hilo(b)
    return np.concatenate([hi, hi, lo, lo], 0)


def build_nc():
    f32 = mybir.dt.float32
    bf16 = mybir.dt.bfloat16
    fp16 = mybir.dt.float16
    MIN = mybir.AluOpType.min
    MULT = mybir.AluOpType.mult

    nc = bacc.Bacc(None)

    bt_d = nc.declare_dram_parameter("bt", [20, NB], bf16, isOutput=False)
    at_d = nc.declare_dram_parameter("at", [20, A_SH], bf16, isOutput=False)
    st_d = nc.declare_dram_parameter("st", [20, S_SH], bf16, isOutput=False)
    qt_d = nc.declare_dram_parameter("qt", [20, N], bf16, isOutput=False)
    rt_d = nc.declare_dram_parameter("rt", [20, M], bf16, isOutput=False)

    colcd_d = nc.declare_dram_parameter("colcd", [128, NT], f32, isOutput=True)
    colseed_d = nc.declare_dram_parameter("colseed", [128, NT], f32, isOutput=True)
    rowcd_d = nc.declare_dram_parameter("rowcd", [128, A_SH], fp16, isOutput=True)
    rowseed_d = nc.declare_dram_parameter("rowseed", [128, S_SH], fp16, isOutput=True)
    confmin_d = nc.declare_dram_parameter("confmin", [128, N // 128], f32, isOutput=True)

    with tile.TileContext(nc) as tc:
        with (
            tc.tile_pool(name="const", bufs=1) as cpool,
            tc.tile_pool(name="evac", bufs=3) as epool,
            tc.tile_pool(name="acc", bufs=1) as apool,
            tc.tile_pool(name="junk", bufs=2) as jpool,
        ):
            bt = cpool.tile([20, NB], bf16, tag="bt")
            nc.sync.dma_start(bt[:], bt_d[:])
            at = cpool.tile([20, A_SH], bf16, tag="at")
            nc.sync.dma_start(at[:], at_d[:])
            st = cpool.tile([20, S_SH], bf16, tag="st")
            nc.sync.dma_start(st[:], st_d[:])
            qt = cpool.tile([20, N], bf16, tag="qt")
            nc.sync.dma_start(qt[:], qt_d[:])
            rt = cpool.tile([20, M], bf16, tag="rt")
            nc.sync.dma_start(rt[:], rt_d[:])

            rowcd = apool.tile([128, A_SH], fp16, tag="rowcd")
            rowseed = apool.tile([128, S_SH], fp16, tag="rowseed")
            colcd = apool.tile([128, NT], f32, tag="colcd")
            colseed = apool.tile([128, NT], f32, tag="colseed")
            confmin = apool.tile([128, N // 128], f32, tag="confmin")
            nc.vector.memset(rowcd[:], 60000.0)
            nc.vector.memset(rowseed[:], 60000.0)

            # Phase 1: cd chamfer. One [128, 2048] psum (4 banks) per b-tile,
            # double-buffered = all 8 banks; single big ACT evac per tile.
            with tc.tile_pool(name="ps1", bufs=2, space="PSUM") as ps1:
                for t in range(NT):
                    lhsT = bt[:, t * 128 : (t + 1) * 128]
                    ps = ps1.tile([128, A_SH], f32, tag="ps")
                    for c in range(4):
                        nc.tensor.matmul(
                            ps[:, c * 512 : (c + 1) * 512],
                            lhsT,
                            at[:, c * 512 : (c + 1) * 512],
                            start=True,
                            stop=True,
                        )
                    ecd = epool.tile([128, A_SH], fp16, tag="ecd")
                    nc.scalar.copy(ecd[:], ps[:])
                    jcd = jpool.tile([128, A_SH], fp16, tag="jcd")
                    nc.vector.tensor_scalar(
                        out=jcd[:], in0=ecd[:], scalar1=1.0, scalar2=None,
                        op0=MULT, op1=MIN, accum_out=colcd[:, t : t + 1],
                    )
                    nc.vector.tensor_tensor(
                        out=rowcd[:], in0=rowcd[:], in1=ecd[:], op=MIN
                    )

            # Phase 2: seed chamfer, 4 b-tiles batched per psum/evac.
            with tc.tile_pool(name="ps2", bufs=2, space="PSUM") as ps2:
                for g in range(NT // 4):
                    ps = ps2.tile([128, 4 * S_SH], f32, tag="ps")
                    for k in range(4):
                        t = g * 4 + k
                        nc.tensor.matmul(
                            ps[:, k * S_SH : (k + 1) * S_SH],
                            bt[:, t * 128 : (t + 1) * 128],
                            st[:],
                            start=True,
                            stop=True,
                        )
                    esd = epool.tile([128, 4 * S_SH], fp16, tag="ecd")
                    nc.scalar.copy(esd[:], ps[:])
                    jsd = jpool.tile([128, S_SH], fp16, tag="jsd")
                    for k in range(4):
                        t = g * 4 + k
                        nc.vector.tensor_scalar(
                            out=jsd[:], in0=esd[:, k * S_SH : (k + 1) * S_SH],
                            scalar1=1.0, scalar2=None,
                            op0=MULT, op1=MIN, accum_out=colseed[:, t : t + 1],
                        )
                    half = epool.tile([128, 2 * S_SH], fp16, tag="ehalf")
                    nc.vector.tensor_tensor(
                        out=half[:], in0=esd[:, : 2 * S_SH], in1=esd[:, 2 * S_SH :],
                        op=MIN,
                    )
                    quar = jpool.tile([128, S_SH], fp16, tag="jsd2")
                    nc.vector.tensor_tensor(
                        out=quar[:], in0=half[:, :S_SH], in1=half[:, S_SH:], op=MIN
                    )
                    nc.vector.tensor_tensor(
                        out=rowseed[:], in0=rowseed[:], in1=quar[:], op=MIN
                    )

                # Phase 3: confidence (reuses ps2 shapes).
                for s in range(N // 128):
                    lhsT = qt[:, s * 128 : (s + 1) * 128]
                    ps = ps2.tile([128, M], f32, tag="ps")
                    for c in range(4):
                        nc.tensor.matmul(
                            ps[:, c * 512 : (c + 1) * 512],
                            lhsT,
                            rt[:, c * 512 : (c + 1) * 512],
                            start=True,
                            stop=True,
                        )
                    ecf = epool.tile([128, M], fp16, tag="ecd")
                    nc.scalar.copy(ecf[:], ps[:])
                    jcf = jpool.tile([128, M], fp16, tag="jcd")
                    nc.vector.tensor_scalar(
                        out=jcf[:], in0=ecf[:], scalar1=1.0, scalar2=None,
                        op0=MULT, op1=MIN, accum_out=confmin[:, s : s + 1],
                    )

            nc.sync.dma_start(colcd_d[:], colcd[:])
            nc.sync.dma_start(colseed_d[:], colseed[:])
            nc.sync.dma_start(rowcd_d[:], rowcd[:])
            nc.sync.dma_start(rowseed_d[:], rowseed[:])
            nc.sync.dma_start(confmin_d[:], confmin[:])

    nc.compile()
    return nc


_NC_CACHE = {}


def _get_nc():
    if "nc" not in _NC_CACHE:
        _NC_CACHE["nc"] = build_nc()
    return _NC_CACHE["nc"]


def run_device(in_maps, trace=False, **kw):
    nc = _get_nc()
    return run_bass_kernel_spmd(nc, in_maps, list(range(N_CORES)), trace=trace, **kw)


def make_in_maps(pc1_0, pc1_1, pc2, pc3):
    a_full = pc1_0.reshape(-1, 3).astype(np.float32)
    s_full = pc1_1.reshape(-1, 3).astype(np.float32)
    b_full = pc2.reshape(-1, 3).astype(np.float32)

    bt = np.ascontiguousarray(_aug_stationary(b_full))
    in_maps = []
    for i in range(N_CORES):
        at = _aug_moving(a_full[i * A_SH : (i + 1) * A_SH])
        st = _aug_moving(s_full[i * S_SH : (i + 1) * S_SH])
        qt = _aug_stationary(pc3[i].astype(np.float32))
        rt = _aug_moving(pc2[i].astype(np.float32))
        in_maps.append(
            {
                "bt": bt,
                "at": np.ascontiguousarray(at),
                "st": np.ascontiguousarray(st),
                "qt": np.ascontiguousarray(qt),
                "rt": np.ascontiguousarray(rt),
            }
        )
    return in_maps


def combine(results, pc1_0, pc1_3, pc2):
    # cd chamfer
    colcd = np.min([r["colcd"] for r in results], axis=0)  # [128, NT]
    d_b = np.sqrt(np.clip(colcd.T.reshape(-1), 0.0, None))  # per-b nearest-a
    rowcd = np.concatenate(
        [r["rowcd"].astype(np.float32).min(0) for r in results]
    )  # [16384] per-a nearest-b
    d_a = np.sqrt(np.clip(rowcd, 0.0, None))
    cd = d_b.mean() + d_a.mean()

    # seed chamfer
    colseed = np.min([r["colseed"] for r in results], axis=0)
    d_b2 = np.sqrt(np.clip(colseed.T.reshape(-1), 0.0, None))
    rowseed = np.concatenate(
        [r["rowseed"].astype(np.float32).min(0) for r in results]
    )
    d_a2 = np.sqrt(np.clip(rowseed, 0.0, None))
    seed = d_b2.mean() + d_a2.mean()

    # confidence
    gts = []
    for r in results:
        cm = r["confmin"].T.reshape(-1)  # [256]
        gts.append(np.exp(-np.sqrt(np.clip(cm, 0.0, None))))
    gt = np.stack(gts)[..., None]  # [8, 256, 1]
    conf_mse = np.mean((pc1_3.astype(np.float32) - gt) ** 2)

    p2p = np.mean((pc1_0.astype(np.float32) - pc2.astype(np.float32)) ** 2)

    loss = ALPHA * cd + BETA * seed + ALPHA * conf_mse + p2p
    return np.array(loss, dtype=np.float32)


def kernel(pc1_0, pc1_1, pc1_3, pc2, pc3):
    in_maps = make_in_maps(pc1_0, pc1_1, pc2, pc3)
    res = run_device(in_maps)
    return combine(res.results, pc1_0, pc1_3, pc2)


if __name__ == "__main__":
    rng = np.random.default_rng(0)
    inputs = {
        "pc1_0": rng.standard_normal((B, M, 3), dtype=np.float32),
        "pc1_1": rng.standard_normal((B, S, 3), dtype=np.float32),
        "pc1_3": rng.random((B, N, 1), dtype=np.float32),
        "pc2": rng.standard_normal((B, M, 3), dtype=np.float32),
        "pc3": rng.standard_normal((B, N, 3), dtype=np.float32),
    }
    print(kernel(**inputs))

